# revision 27
# baseline (speedup 1.0000x reference)
"""CompressedFP8Linear on 8 trn2 NeuronCores.

out[B,S,O] = x @ (weight * weight_scale).T + bias
  x:[4,32,8192] f32, weight:[8192,8192] f32 (fp8-e4m3 representable),
  weight_scale:[8192,1] f32, bias:[8192] f16.

Column-parallel (per sharding hint): weight rows (out_features) sharded
8 ways, x replicated, host concatenates the output shards.

Per-core dataflow (DMA floor ~10.3 MiB -> ~29 us at 360 GB/s):
  - weight ships as fp8-e4m3 (exact: the reference round-trips through
    fp8), 8 MiB/core; x ships fp16 (2 MiB); out returns fp16 (0.25 MiB).
  - Input stream striped across BOTH HWDGE queues (scalar+sync) so DGE
    config overhead pipelines; output DMAs ride gpsimd SWDGE so they
    never block the next iteration's fill.
  - The out_features half is processed as two sequential column groups:
    og0's k-stream completes mid-kernel, so its scale-mul + store hide
    under og1's stream; only og1's short taper is tail-exposed.
  - bias is pre-divided by scale on host and injected as the PSUM
    accumulation seed via a ones-outer-product matmul (start=True), so
    the epilogue is a single DVE mul per group: out = acc * scale.
  - A chain of tiny warmup matmuls at kernel start ramps the PE clock
    (0.65 -> 2.4 GHz takes ~3 us of continuous activity) while the DMA
    fill runs, so real matmuls start at full rate.
  - scale/bias rows ship fp16 (~3e-4 rel err, budget 2e-2) and are
    broadcast to the 128 token partitions via exact ones-outer-products
    on the PE; fp16 moving operands stream at 1 col/cycle (fp32 would
    be 4x slower).
"""

import numpy as np
import ml_dtypes

import concourse.bass as bass
import concourse.mybir as mybir
import concourse.tile as tile
from concourse.bass_utils import run_bass_kernel_spmd

B, S, IN, OUT = 4, 32, 8192, 8192
M = B * S                      # 128 tokens
NCORES = 8
OSH = OUT // NCORES            # 1024 out-features per core
KT = IN // 128                 # 64 k-tiles
OG = 512                       # columns per output group (PSUM bank width)
F32 = mybir.dt.float32
F16 = mybir.dt.float16
F8 = mybir.dt.float8e4         # numpy side: ml_dtypes.float8_e4m3

# k-tile slab schedule: both output halves interleave per k-tile (PE
# eats a k-tile in 2 matmuls, ~1.17x the DMA delivery rate, so the
# stream stays ahead).  First slabs small so matmul 0 starts fast; big
# middle slabs for DMA efficiency; tapered end so the final
# data->matmul->store chain is short.
W_SLABS = [(0, 2), (2, 2), (4, 4), (8, 8), (16, 8), (24, 8), (32, 8),
           (40, 8), (48, 8), (56, 4), (60, 2), (62, 1), (63, 1)]
MAX_SLAB = 8


def split_waits(nc, max_waits=1):
    """This walrus build encodes at most one sem-wait per instruction;
    move any excess onto NoOps injected just before (same engine queue,
    so ordering semantics are identical)."""
    n = 0
    for f in nc.m.functions:
        for bb in f.blocks:
            out = []
            for inst in bb.instructions:
                si = inst.sync_info
                waits = list(si.on_wait) if si and si.on_wait else []
                if len(waits) > max_waits:
                    extra, keep = waits[:-max_waits], waits[-max_waits:]
                    for i, w in enumerate(extra):
                        out.append(mybir.InstNoOp(
                            name=f"{inst.name}-ws{i}", engine=inst.engine,
                            ins=[], outs=[],
                            sync_info=mybir.SyncInfo(on_wait=[w], on_update=[])))
                        n += 1
                    si.on_wait = keep
                out.append(inst)
            bb.instructions = out
    return n


def _declare(nc):
    # host-packed layouts: each SBUF partition's DMA is one contiguous run
    xt_d = nc.dram_tensor("xt", [128, KT, M], F16, kind="ExternalInput")
    wt_d = nc.dram_tensor("wt", [128, KT, OSH], F8, kind="ExternalInput")
    # rows = [scale_row | bias_over_scale_row], one 4KB DMA
    rows_d = nc.dram_tensor("rows", [1, 2 * OSH], F16, kind="ExternalInput")
    out_d = nc.dram_tensor("out", [M, OSH], F16, kind="ExternalOutput")
    return xt_d, wt_d, rows_d, out_d


def _emit_rep(nc, pools, queues, tensors, ones, brow, sc, single_q=0):
    """One full shard computation: out[128, OSH] = (x @ W^T + bias/sc)*sc."""
    xp, wp, op, ps = pools
    xt_d, wt_d, out_d = tensors

    xsb = xp.tile([128, KT, M], F16)

    # separate tags so consecutive reps alternate over 4 PSUM banks
    acc0 = ps.tile([M, OG], F32, tag="a0")
    acc1 = ps.tile([M, OG], F32, tag="a1")
    accs = (acc0, acc1)
    for g in range(2):
        # seed PSUM with bias/scale (exact fp16 ones outer product)
        nc.tensor.matmul(accs[g][:, :], ones[:, :],
                         brow[:, g * OG:(g + 1) * OG],
                         start=True, stop=False)
    # w slabs stripe across both HWDGE queues; x (4x smaller) rides two
    # slabs ahead on the same queue so it always lands before the
    # (other-queue, concurrent) w slab that needs it.
    k0, n = W_SLABS[0]
    xk = W_SLABS[2][0] + W_SLABS[2][1]   # x for slabs 0..2 upfront
    queues[0 if single_q else 1].dma_start(xsb[:, :xk, :], xt_d[:, :xk, :])
    for i, (k0, n) in enumerate(W_SLABS):
        q = queues[0 if single_q else i % 2]
        wsb = wp.tile([128, MAX_SLAB, OSH], F8, tag="wsb")
        q.dma_start(wsb[:, :n, :], wt_d[:, k0:k0 + n, :])
        if i + 3 < len(W_SLABS):
            nk0, nn = W_SLABS[i + 3]
            q.dma_start(xsb[:, nk0:nk0 + nn, :], xt_d[:, nk0:nk0 + nn, :])
        for s in range(n):
            k = k0 + s
            for g in range(2):
                nc.tensor.matmul(accs[g][:, :], xsb[:, k, :],
                                 wsb[:, s, g * OG:(g + 1) * OG],
                                 start=False, stop=(k == KT - 1))
    for g in range(2):
        osl = slice(g * OG, (g + 1) * OG)
        outsb = op.tile([M, OG], F16, tag=f"o{g}")
        nc.vector.tensor_mul(outsb[:], accs[g][:, :], sc[:, osl])
        queues[1 if single_q else g].dma_start(out_d[:, osl], outsb[:])


def build(reps=1, loops=0, warm=28, single_q=0):
    """One column-parallel shard.

    reps > 1 unrolls the whole body back-to-back (steady-state timing);
    loops > 0 wraps the reps in a hardware For_i loop (low-noise
    on-device timing; every iteration recomputes the same output).
    """
    nc = bass.Bass()
    xt_d, wt_d, rows_d, out_d = _declare(nc)

    with tile.TileContext(nc) as tc:
        with (
            tc.tile_pool(name="xp", bufs=2) as xp,
            tc.tile_pool(name="wp", bufs=8) as wp,
            tc.tile_pool(name="cp", bufs=1) as cp,
            tc.tile_pool(name="op", bufs=2) as op,
            tc.tile_pool(name="pw", bufs=1, space="PSUM") as pw,
            tc.tile_pool(name="psb", bufs=1, space="PSUM") as psb,
            tc.tile_pool(name="ps", bufs=2, space="PSUM") as ps,
        ):
            queues = (nc.scalar, nc.sync)

            # constants: ones column, scale/bias rows (tiny, on sync queue)
            ones = cp.tile([1, M], F16, tag="ones")
            nc.vector.memset(ones[:], 1.0)
            # rows ride gpsimd SWDGE so they don't clog the HWDGE queue
            # heads ahead of the first w/x stream jobs
            rows = cp.tile([1, 2 * OSH], F16, tag="rows")
            nc.gpsimd.dma_start(rows[:], rows_d[:])
            srow = rows[:, :OSH]
            brow = rows[:, OSH:]

            # PE clock warmup: tiny back-to-back matmuls (same-engine
            # program order, no semaphores) keep the PE continuously busy
            # through the ~3us ramp while the first DMAs land.
            warm_ps = pw.tile([1, OG], F32)
            for _ in range(warm):
                nc.tensor.matmul(warm_ps[:, :64], ones[:, :1], ones[:, :64],
                                 start=True, stop=True)

            # broadcast scale to all 128 token partitions (once; exact)
            pb = psb.tile([M, OSH], F32)
            for g in range(2):
                nc.tensor.matmul(pb[:, g * OG:(g + 1) * OG], ones[:, :],
                                 srow[:, g * OG:(g + 1) * OG],
                                 start=True, stop=True)
            sc = cp.tile([M, OSH], F32, tag="scbc")
            nc.vector.tensor_copy(sc[:], pb[:])

            pools = (xp, wp, op, ps)
            tensors = (xt_d, wt_d, out_d)

            def emit_reps():
                for _ in range(reps):
                    _emit_rep(nc, pools, queues, tensors, ones, brow, sc,
                              single_q=single_q)

            if loops > 0:
                with tc.For_i(0, loops):
                    emit_reps()
            else:
                emit_reps()

    split_waits(nc)
    return nc


def shard_inputs(x, weight, weight_scale, bias):
    """Host-side marshalling into per-core input maps (layout + dtype only;
    the fp8 weight conversion is exact because the reference round-trips
    weight through fp8-e4m3)."""
    x = np.asarray(x, dtype=np.float32)
    weight = np.asarray(weight, dtype=np.float32)
    scale = np.asarray(weight_scale, dtype=np.float32).reshape(OUT)
    bias32 = np.asarray(bias).astype(np.float32)

    # pack x as [p, kt, m] (k = kt*128 + p) so each SBUF partition's x data
    # is one contiguous DRAM run
    xt = np.ascontiguousarray(
        np.transpose(x.reshape(M, KT, 128), (2, 1, 0))).astype(np.float16)
    in_maps = []
    for c in range(NCORES):
        sl = slice(c * OSH, (c + 1) * OSH)
        # wt[p, kt, o] = W_shard[o, kt*128 + p]  (k-major on partitions)
        wt = np.ascontiguousarray(
            weight[sl, :].T.reshape(KT, 128, OSH).transpose(1, 0, 2)
        ).astype(ml_dtypes.float8_e4m3)
        rows = np.concatenate(
            [scale[sl], bias32[sl] / scale[sl]])[None, :].astype(np.float16)
        in_maps.append({
            "xt": xt, "wt": wt, "rows": np.ascontiguousarray(rows),
        })
    return in_maps


def kernel(x, weight, weight_scale, bias):
    nc = build(reps=1)
    in_maps = shard_inputs(x, weight, weight_scale, bias)
    res = run_bass_kernel_spmd(nc, in_maps, core_ids=list(range(NCORES)))
    out = np.concatenate(
        [np.asarray(res.results[c]["out"]) for c in range(NCORES)], axis=1)
    return out.astype(np.float32).reshape(B, S, OUT)


# revision 43
# speedup vs baseline: 1.3858x; 1.3858x over previous
"""CompressedFP8Linear on 8 trn2 NeuronCores.

out[B,S,O] = x @ (weight * weight_scale).T + bias
  x:[4,32,8192] f32, weight:[8192,8192] f32 (fp8-e4m3 representable),
  weight_scale:[8192,1] f32, bias:[8192] f16.

Column-parallel (per sharding hint): weight rows (out_features) sharded
8 ways, x replicated, host concatenates the output shards.

The kernel is memory-bound at the CHIP level: with all 8 cores
streaming, per-core DMA sits at ~340-360 GB/s (measured 500-850 GB/s
single-core, so the shared HBM is the wall).  Per-core traffic is the
floor: 8 MiB weight (fp8, exact since the reference round-trips through
e4m3) + 2 MiB x (fp16) + 0.25 MiB out (fp16) ~= 29-31 us steady state.

Structure:
  - w streams in 4-ktile slabs (4 KiB descriptors) striped across BOTH
    HWDGE queues (scalar+sync); x rides in four 16-ktile chunks (4 KiB
    descriptors) paced two slabs ahead of consumption.  Fine slab
    granularity minimizes PE wait per slab (single-shot makespan);
    chunked x keeps job count low (steady-state rate).
  - bias is pre-divided by scale on host, shipped fp16, and injected as
    the PSUM seed via a ones-outer-product matmul (start=True), so the
    epilogue is one DVE mul per 512-col group: out = acc * scale_bcast.
  - scale row is broadcast to the 128 token partitions once via exact
    ones-outer-products on the PE (fp16 moving = 1 col/cycle; fp32
    would stream 4x slower).
  - a chain of tiny warmup matmuls bridges the PE clock ramp
    (0.65 -> 2.4 GHz over ~3 us of continuous activity) while the
    first DMAs land, and doubles as the wait for the scale/bias row.
  - in the final 8 ktiles, og0's matmuls run before og1's so og0's
    scale-mul + store overlap og1's tail matmuls.
  - scale/bias rows ride gpsimd SWDGE so they never clog the HWDGE
    queue heads; outputs go out per 512-col group on the two HWDGE
    queues as soon as each group's mul completes.

Measured (8 cores, loop-amplified slopes): ~34 us marginal per-rep
steady state, ~44 us per barrier-separated single-shot iteration
(machine-state drift +/-1.5 us; fp32 v0 baseline measured 34.5-35 us
steady / ~47 us single-shot on the same days).  Relative error 3.7e-4
(budget 2e-2).
"""

import numpy as np
import ml_dtypes

import concourse.bass as bass
import concourse.mybir as mybir
import concourse.tile as tile
from concourse.bass_utils import run_bass_kernel_spmd

B, S, IN, OUT = 4, 32, 8192, 8192
M = B * S                      # 128 tokens
NCORES = 8
OSH = OUT // NCORES            # 1024 out-features per core
KT = IN // 128                 # 64 k-tiles
OG = 512                       # columns per output group (PSUM bank width)
F32 = mybir.dt.float32
F16 = mybir.dt.float16
F8 = mybir.dt.float8e4         # numpy side: ml_dtypes.float8_e4m3

# k-tile slab schedule: both output halves interleave per k-tile (PE
# eats a k-tile in 2 matmuls, ~1.17x the DMA delivery rate, so the
# stream stays ahead).  First slabs small so matmul 0 starts fast; big
# middle slabs for DMA efficiency; tapered end so the final
# data->matmul->store chain is short.
W_SLABS = [(0, 2), (2, 2), (4, 4), (8, 8), (16, 8), (24, 8), (32, 8),
           (40, 8), (48, 8), (56, 4), (60, 2), (62, 1), (63, 1)]
MAX_SLAB = 8


def split_waits(nc, max_waits=1):
    """This walrus build encodes at most one sem-wait per instruction;
    move any excess onto NoOps injected just before (same engine queue,
    so ordering semantics are identical)."""
    n = 0
    for f in nc.m.functions:
        for bb in f.blocks:
            out = []
            for inst in bb.instructions:
                si = inst.sync_info
                waits = list(si.on_wait) if si and si.on_wait else []
                if len(waits) > max_waits:
                    extra, keep = waits[:-max_waits], waits[-max_waits:]
                    for i, w in enumerate(extra):
                        out.append(mybir.InstNoOp(
                            name=f"{inst.name}-ws{i}", engine=inst.engine,
                            ins=[], outs=[],
                            sync_info=mybir.SyncInfo(on_wait=[w], on_update=[])))
                        n += 1
                    si.on_wait = keep
                out.append(inst)
            bb.instructions = out
    return n


def _declare(nc):
    # host-packed layouts: each SBUF partition's DMA is one contiguous run
    xt_d = nc.dram_tensor("xt", [128, KT, M], F16, kind="ExternalInput")
    wt_d = nc.dram_tensor("wt", [128, KT, OSH], F8, kind="ExternalInput")
    # rows = [scale_row | bias_over_scale_row], one 4KB DMA
    rows_d = nc.dram_tensor("rows", [1, 2 * OSH], F16, kind="ExternalInput")
    out_d = nc.dram_tensor("out", [M, OSH], F16, kind="ExternalOutput")
    return xt_d, wt_d, rows_d, out_d


def _emit_rep(nc, pools, queues, tensors, ones, brow, sc, single_q=0,
              half_mm=0, x_mode=0, w_mode=0, out_mode=0, out_eng=None):
    n_g = 0 if half_mm == 2 else (1 if half_mm else 2)
    if w_mode == 1:
        slabs = [(k, 8) for k in range(0, KT, 8)]
    elif w_mode >= 2:
        mid = {2: 12, 3: 16, 4: 4, 5: 2}[w_mode]
        slabs = [(0, 2), (2, 2), (4, 4)]
        k = 8
        while k < KT - 8:
            n = min(mid, KT - 8 - k)
            slabs.append((k, n))
            k += n
        slabs += [(KT - 8, 4), (KT - 4, 2), (KT - 2, 1), (KT - 1, 1)]
    else:
        slabs = W_SLABS
    """One full shard computation: out[128, OSH] = (x @ W^T + bias/sc)*sc."""
    xp, wp, op, ps = pools
    xt_d, wt_d, out_d = tensors

    xsb = xp.tile([128, KT, M], F16)

    # separate tags so consecutive reps alternate over 4 PSUM banks
    acc0 = ps.tile([M, OG], F32, tag="a0")
    acc1 = ps.tile([M, OG], F32, tag="a1")
    accs = (acc0, acc1)
    for g in range(2):
        # seed PSUM with bias/scale (exact fp16 ones outer product)
        nc.tensor.matmul(accs[g][:, :], ones[:, :],
                         brow[:, g * OG:(g + 1) * OG],
                         start=True, stop=(half_mm == 2))
    # w slabs stripe across both HWDGE queues; x (4x smaller) rides two
    # slabs ahead on the same queue so it always lands before the
    # (other-queue, concurrent) w slab that needs it.
    taper_bufs = []
    x0q = 0 if (single_q or out_mode == 2) else 1
    if x_mode == 1:
        # two 32-kt chunks: 8KB descriptors
        queues[x0q].dma_start(xsb[:, :32, :], xt_d[:, :32, :])
    elif x_mode == 2:
        # four 16-kt chunks (4KB descs): first upfront, rest paced below
        queues[x0q].dma_start(xsb[:, :16, :], xt_d[:, :16, :])
    else:
        xk = slabs[2][0] + slabs[2][1]   # x for slabs 0..2 upfront
        queues[x0q].dma_start(xsb[:, :xk, :], xt_d[:, :xk, :])
    for i, (k0, n) in enumerate(slabs):
        q = queues[0 if single_q else i % 2]
        wsb = wp.tile([128, max(x[1] for x in slabs), OSH], F8, tag="wsb")
        q.dma_start(wsb[:, :n, :], wt_d[:, k0:k0 + n, :])
        if x_mode == 1:
            if i == 1:
                queues[0 if single_q else 0].dma_start(
                    xsb[:, 32:, :], xt_d[:, 32:, :])
        elif x_mode == 2:
            if k0 in (8, 24, 40) and k0 + n <= 56:
                a = k0 + 8
                q.dma_start(xsb[:, a:a + 16, :], xt_d[:, a:a + 16, :])
        elif i + 3 < len(slabs):
            nk0, nn = slabs[i + 3]
            q.dma_start(xsb[:, nk0:nk0 + nn, :], xt_d[:, nk0:nk0 + nn, :])
        if k0 >= KT - 8:
            taper_bufs.append((k0, n, wsb))
            continue
        for s in range(n):
            k = k0 + s
            for g in range(n_g):
                nc.tensor.matmul(accs[g][:, :], xsb[:, k, :],
                                 wsb[:, s, g * OG:(g + 1) * OG],
                                 start=False, stop=False)

    def emit_epilogue(g):
        osl = slice(g * OG, (g + 1) * OG)
        outsb = op.tile([M, OG], F16, tag=f"o{g}")
        nc.vector.tensor_mul(outsb[:], accs[0 if half_mm else g][:, :],
                             sc[:, osl])
        oq = queues[1] if out_mode == 2 else queues[1 if single_q else g]
        oq.dma_start(out_d[:, osl], outsb[:])

    # taper: finish og0's accumulation first so its scale-mul + store
    # overlap og1's final matmuls instead of serializing after them
    for g in range(n_g):
        for k0, n, wsb in taper_bufs:
            for s in range(n):
                k = k0 + s
                nc.tensor.matmul(accs[g][:, :], xsb[:, k, :],
                                 wsb[:, s, g * OG:(g + 1) * OG],
                                 start=False, stop=(k == KT - 1))
        emit_epilogue(g)
    if n_g == 0:
        emit_epilogue(0)
        emit_epilogue(1)
    elif n_g == 1:
        emit_epilogue(1)


def build(reps=1, loops=0, warm=28, single_q=0, half_mm=0, x_mode=2, w_mode=4, wp_bufs=8, out_mode=0):
    """One column-parallel shard.

    reps > 1 unrolls the whole body back-to-back (steady-state timing);
    loops > 0 wraps the reps in a hardware For_i loop (low-noise
    on-device timing; every iteration recomputes the same output).
    """
    nc = bass.Bass()
    xt_d, wt_d, rows_d, out_d = _declare(nc)

    with tile.TileContext(nc) as tc:
        with (
            tc.tile_pool(name="xp", bufs=2) as xp,
            tc.tile_pool(name="wp", bufs=wp_bufs) as wp,
            tc.tile_pool(name="cp", bufs=1) as cp,
            tc.tile_pool(name="op", bufs=2) as op,
            tc.tile_pool(name="pw", bufs=1, space="PSUM") as pw,
            tc.tile_pool(name="psb", bufs=1, space="PSUM") as psb,
            tc.tile_pool(name="ps", bufs=2, space="PSUM") as ps,
        ):
            queues = (nc.scalar, nc.sync)

            # constants: ones column, scale/bias rows (tiny, on sync queue)
            ones = cp.tile([1, M], F16, tag="ones")
            nc.vector.memset(ones[:], 1.0)
            # rows ride gpsimd SWDGE so they don't clog the HWDGE queue
            # heads ahead of the first w/x stream jobs
            rows = cp.tile([1, 2 * OSH], F16, tag="rows")
            nc.gpsimd.dma_start(rows[:], rows_d[:])
            srow = rows[:, :OSH]
            brow = rows[:, OSH:]

            # PE clock warmup: tiny back-to-back matmuls (same-engine
            # program order, no semaphores) keep the PE continuously busy
            # through the ~3us ramp while the first DMAs land.
            warm_ps = pw.tile([1, OG], F32)
            for _ in range(warm):
                nc.tensor.matmul(warm_ps[:, :64], ones[:, :1], ones[:, :64],
                                 start=True, stop=True)

            # broadcast scale to all 128 token partitions (once; exact)
            pb = psb.tile([M, OSH], F32)
            for g in range(2):
                nc.tensor.matmul(pb[:, g * OG:(g + 1) * OG], ones[:, :],
                                 srow[:, g * OG:(g + 1) * OG],
                                 start=True, stop=True)
            sc = cp.tile([M, OSH], F32, tag="scbc")
            nc.vector.tensor_copy(sc[:], pb[:])

            pools = (xp, wp, op, ps)
            tensors = (xt_d, wt_d, out_d)

            def emit_reps():
                for _ in range(reps):
                    _emit_rep(nc, pools, queues, tensors, ones, brow, sc,
                              single_q=single_q, half_mm=half_mm,
                              x_mode=x_mode, w_mode=w_mode,
                              out_mode=out_mode, out_eng=nc.gpsimd)

            if loops > 0:
                with tc.For_i(0, loops):
                    emit_reps()
            else:
                emit_reps()

    split_waits(nc)
    return nc


def shard_inputs(x, weight, weight_scale, bias):
    """Host-side marshalling into per-core input maps (layout + dtype only;
    the fp8 weight conversion is exact because the reference round-trips
    weight through fp8-e4m3)."""
    x = np.asarray(x, dtype=np.float32)
    weight = np.asarray(weight, dtype=np.float32)
    scale = np.asarray(weight_scale, dtype=np.float32).reshape(OUT)
    bias32 = np.asarray(bias).astype(np.float32)

    # pack x as [p, kt, m] (k = kt*128 + p) so each SBUF partition's x data
    # is one contiguous DRAM run
    xt = np.ascontiguousarray(
        np.transpose(x.reshape(M, KT, 128), (2, 1, 0))).astype(np.float16)
    in_maps = []
    for c in range(NCORES):
        sl = slice(c * OSH, (c + 1) * OSH)
        # wt[p, kt, o] = W_shard[o, kt*128 + p]  (k-major on partitions)
        wt = np.ascontiguousarray(
            weight[sl, :].T.reshape(KT, 128, OSH).transpose(1, 0, 2)
        ).astype(ml_dtypes.float8_e4m3)
        rows = np.concatenate(
            [scale[sl], bias32[sl] / scale[sl]])[None, :].astype(np.float16)
        in_maps.append({
            "xt": xt, "wt": wt, "rows": np.ascontiguousarray(rows),
        })
    return in_maps


def kernel(x, weight, weight_scale, bias):
    nc = build(reps=1)
    in_maps = shard_inputs(x, weight, weight_scale, bias)
    res = run_bass_kernel_spmd(nc, in_maps, core_ids=list(range(NCORES)))
    out = np.concatenate(
        [np.asarray(res.results[c]["out"]) for c in range(NCORES)], axis=1)
    return out.astype(np.float32).reshape(B, S, OUT)


# revision 44
# speedup vs baseline: 1.3983x; 1.0090x over previous
"""CompressedFP8Linear on 8 trn2 NeuronCores.

out[B,S,O] = x @ (weight * weight_scale).T + bias
  x:[4,32,8192] f32, weight:[8192,8192] f32 (fp8-e4m3 representable),
  weight_scale:[8192,1] f32, bias:[8192] f16.

Column-parallel (per sharding hint): weight rows (out_features) sharded
8 ways, x replicated, host concatenates the output shards.

The kernel is memory-bound at the CHIP level: with all 8 cores
streaming, per-core DMA sits at ~340-360 GB/s (measured 500-850 GB/s
single-core, so the shared HBM is the wall).  Per-core traffic is the
floor: 8 MiB weight (fp8, exact since the reference round-trips through
e4m3) + 2 MiB x (fp16) + 0.25 MiB out (fp16) ~= 29-31 us steady state.

Structure:
  - w streams in 4-ktile slabs (4 KiB descriptors) striped across BOTH
    HWDGE queues (scalar+sync); x rides in four 16-ktile chunks (4 KiB
    descriptors) paced two slabs ahead of consumption.  Fine slab
    granularity minimizes PE wait per slab (single-shot makespan);
    chunked x keeps job count low (steady-state rate).
  - bias is pre-divided by scale on host, shipped fp16, and injected as
    the PSUM seed via a ones-outer-product matmul (start=True), so the
    epilogue is one DVE mul per 512-col group: out = acc * scale_bcast.
  - scale row is broadcast to the 128 token partitions once via exact
    ones-outer-products on the PE (fp16 moving = 1 col/cycle; fp32
    would stream 4x slower).
  - a chain of tiny warmup matmuls bridges the PE clock ramp
    (0.65 -> 2.4 GHz over ~3 us of continuous activity) while the
    first DMAs land, and doubles as the wait for the scale/bias row.
  - in the final 8 ktiles, og0's matmuls run before og1's so og0's
    scale-mul + store overlap og1's tail matmuls.
  - scale/bias rows ride gpsimd SWDGE so they never clog the HWDGE
    queue heads; outputs go out per 512-col group on the two HWDGE
    queues as soon as each group's mul completes.

Measured (8 cores, loop-amplified slopes): ~34 us marginal per-rep
steady state, ~44 us per barrier-separated single-shot iteration
(machine-state drift +/-1.5 us; fp32 v0 baseline measured 34.5-35 us
steady / ~47 us single-shot on the same days).  Relative error 3.7e-4
(budget 2e-2).
"""

import numpy as np
import ml_dtypes

import concourse.bass as bass
import concourse.mybir as mybir
import concourse.tile as tile
from concourse.bass_utils import run_bass_kernel_spmd

B, S, IN, OUT = 4, 32, 8192, 8192
M = B * S                      # 128 tokens
NCORES = 8
OSH = OUT // NCORES            # 1024 out-features per core
KT = IN // 128                 # 64 k-tiles
OG = 512                       # columns per output group (PSUM bank width)
F32 = mybir.dt.float32
F16 = mybir.dt.float16
F8 = mybir.dt.float8e4         # numpy side: ml_dtypes.float8_e4m3

# k-tile slab schedule: both output halves interleave per k-tile (PE
# eats a k-tile in 2 matmuls, ~1.17x the DMA delivery rate, so the
# stream stays ahead).  First slabs small so matmul 0 starts fast; big
# middle slabs for DMA efficiency; tapered end so the final
# data->matmul->store chain is short.
W_SLABS = [(0, 2), (2, 2), (4, 4), (8, 8), (16, 8), (24, 8), (32, 8),
           (40, 8), (48, 8), (56, 4), (60, 2), (62, 1), (63, 1)]
MAX_SLAB = 8


def split_waits(nc, max_waits=1):
    """This walrus build encodes at most one sem-wait per instruction;
    move any excess onto NoOps injected just before (same engine queue,
    so ordering semantics are identical)."""
    n = 0
    for f in nc.m.functions:
        for bb in f.blocks:
            out = []
            for inst in bb.instructions:
                si = inst.sync_info
                waits = list(si.on_wait) if si and si.on_wait else []
                if len(waits) > max_waits:
                    extra, keep = waits[:-max_waits], waits[-max_waits:]
                    for i, w in enumerate(extra):
                        out.append(mybir.InstNoOp(
                            name=f"{inst.name}-ws{i}", engine=inst.engine,
                            ins=[], outs=[],
                            sync_info=mybir.SyncInfo(on_wait=[w], on_update=[])))
                        n += 1
                    si.on_wait = keep
                out.append(inst)
            bb.instructions = out
    return n


def _declare(nc):
    # host-packed layouts: each SBUF partition's DMA is one contiguous run
    xt_d = nc.dram_tensor("xt", [128, KT, M], F16, kind="ExternalInput")
    wt_d = nc.dram_tensor("wt", [128, KT, OSH], F8, kind="ExternalInput")
    # rows = [scale_row | bias_over_scale_row], one 4KB DMA
    rows_d = nc.dram_tensor("rows", [1, 2 * OSH], F16, kind="ExternalInput")
    out_d = nc.dram_tensor("out", [M, OSH], F16, kind="ExternalOutput")
    return xt_d, wt_d, rows_d, out_d


def _emit_rep(nc, pools, queues, tensors, ones, brow, sc, single_q=0,
              half_mm=0, x_mode=0, w_mode=0, out_mode=0, out_eng=None):
    n_g = 0 if half_mm == 2 else (1 if half_mm else 2)
    if w_mode == 1:
        slabs = [(k, 8) for k in range(0, KT, 8)]
    elif w_mode >= 2:
        mid = {2: 12, 3: 16, 4: 4, 5: 2}[w_mode]
        slabs = [(0, 2), (2, 2), (4, 4)]
        k = 8
        while k < KT - 8:
            n = min(mid, KT - 8 - k)
            slabs.append((k, n))
            k += n
        slabs += [(KT - 8, 4), (KT - 4, 2), (KT - 2, 1), (KT - 1, 1)]
    else:
        slabs = W_SLABS
    """One full shard computation: out[128, OSH] = (x @ W^T + bias/sc)*sc."""
    xp, wp, op, ps = pools
    xt_d, wt_d, out_d = tensors

    xsb = xp.tile([128, KT, M], F16)

    # separate tags so consecutive reps alternate over 4 PSUM banks
    acc0 = ps.tile([M, OG], F32, tag="a0")
    acc1 = ps.tile([M, OG], F32, tag="a1")
    accs = (acc0, acc1)
    for g in range(2):
        # seed PSUM with bias/scale (exact fp16 ones outer product)
        nc.tensor.matmul(accs[g][:, :], ones[:, :],
                         brow[:, g * OG:(g + 1) * OG],
                         start=True, stop=(half_mm == 2))
    # w slabs stripe across both HWDGE queues; x (4x smaller) rides two
    # slabs ahead on the same queue so it always lands before the
    # (other-queue, concurrent) w slab that needs it.
    taper_bufs = []
    x0q = 0 if (single_q or out_mode == 2) else 1
    if x_mode == 1:
        # two 32-kt chunks: 8KB descriptors
        queues[x0q].dma_start(xsb[:, :32, :], xt_d[:, :32, :])
    elif x_mode == 2:
        # four 16-kt chunks (4KB descs): first upfront, rest paced below
        queues[x0q].dma_start(xsb[:, :16, :], xt_d[:, :16, :])
    else:
        xk = slabs[2][0] + slabs[2][1]   # x for slabs 0..2 upfront
        queues[x0q].dma_start(xsb[:, :xk, :], xt_d[:, :xk, :])
    for i, (k0, n) in enumerate(slabs):
        q = queues[0 if single_q else i % 2]
        wsb = wp.tile([128, max(x[1] for x in slabs), OSH], F8, tag="wsb")
        q.dma_start(wsb[:, :n, :], wt_d[:, k0:k0 + n, :])
        if x_mode == 1:
            if i == 1:
                queues[0 if single_q else 0].dma_start(
                    xsb[:, 32:, :], xt_d[:, 32:, :])
        elif x_mode == 2:
            if k0 in (8, 24, 40) and k0 + n <= 56:
                a = k0 + 8
                q.dma_start(xsb[:, a:a + 16, :], xt_d[:, a:a + 16, :])
        elif i + 3 < len(slabs):
            nk0, nn = slabs[i + 3]
            q.dma_start(xsb[:, nk0:nk0 + nn, :], xt_d[:, nk0:nk0 + nn, :])
        if k0 >= KT - 8:
            taper_bufs.append((k0, n, wsb))
            continue
        for s in range(n):
            k = k0 + s
            for g in range(n_g):
                nc.tensor.matmul(accs[g][:, :], xsb[:, k, :],
                                 wsb[:, s, g * OG:(g + 1) * OG],
                                 start=False, stop=False)

    def emit_epilogue(g):
        osl = slice(g * OG, (g + 1) * OG)
        outsb = op.tile([M, OG], F16, tag=f"o{g}")
        nc.vector.tensor_mul(outsb[:], accs[0 if half_mm else g][:, :],
                             sc[:, osl])
        oq = queues[1] if out_mode == 2 else queues[1 if single_q else g]
        oq.dma_start(out_d[:, osl], outsb[:])

    # taper: finish og0's accumulation first so its scale-mul + store
    # overlap og1's final matmuls instead of serializing after them
    for g in range(n_g):
        for k0, n, wsb in taper_bufs:
            for s in range(n):
                k = k0 + s
                nc.tensor.matmul(accs[g][:, :], xsb[:, k, :],
                                 wsb[:, s, g * OG:(g + 1) * OG],
                                 start=False, stop=(k == KT - 1))
        emit_epilogue(g)
    if n_g == 0:
        emit_epilogue(0)
        emit_epilogue(1)
    elif n_g == 1:
        emit_epilogue(1)


def build(reps=1, loops=0, warm=28, single_q=0, half_mm=0, x_mode=2, w_mode=4, wp_bufs=8, out_mode=0, xp_bufs=2, op_bufs=2):
    """One column-parallel shard.

    reps > 1 unrolls the whole body back-to-back (steady-state timing);
    loops > 0 wraps the reps in a hardware For_i loop (low-noise
    on-device timing; every iteration recomputes the same output).
    """
    nc = bass.Bass()
    xt_d, wt_d, rows_d, out_d = _declare(nc)

    with tile.TileContext(nc) as tc:
        with (
            tc.tile_pool(name="xp", bufs=xp_bufs) as xp,
            tc.tile_pool(name="wp", bufs=wp_bufs) as wp,
            tc.tile_pool(name="cp", bufs=1) as cp,
            tc.tile_pool(name="op", bufs=op_bufs) as op,
            tc.tile_pool(name="pw", bufs=1, space="PSUM") as pw,
            tc.tile_pool(name="psb", bufs=1, space="PSUM") as psb,
            tc.tile_pool(name="ps", bufs=2, space="PSUM") as ps,
        ):
            queues = (nc.scalar, nc.sync)

            # constants: ones column, scale/bias rows (tiny, on sync queue)
            ones = cp.tile([1, M], F16, tag="ones")
            nc.vector.memset(ones[:], 1.0)
            # rows ride gpsimd SWDGE so they don't clog the HWDGE queue
            # heads ahead of the first w/x stream jobs
            rows = cp.tile([1, 2 * OSH], F16, tag="rows")
            nc.gpsimd.dma_start(rows[:], rows_d[:])
            srow = rows[:, :OSH]
            brow = rows[:, OSH:]

            # PE clock warmup: tiny back-to-back matmuls (same-engine
            # program order, no semaphores) keep the PE continuously busy
            # through the ~3us ramp while the first DMAs land.
            warm_ps = pw.tile([1, OG], F32)
            for _ in range(warm):
                nc.tensor.matmul(warm_ps[:, :64], ones[:, :1], ones[:, :64],
                                 start=True, stop=True)

            # broadcast scale to all 128 token partitions (once; exact)
            pb = psb.tile([M, OSH], F32)
            for g in range(2):
                nc.tensor.matmul(pb[:, g * OG:(g + 1) * OG], ones[:, :],
                                 srow[:, g * OG:(g + 1) * OG],
                                 start=True, stop=True)
            sc = cp.tile([M, OSH], F32, tag="scbc")
            nc.vector.tensor_copy(sc[:], pb[:])

            pools = (xp, wp, op, ps)
            tensors = (xt_d, wt_d, out_d)

            def emit_reps():
                for _ in range(reps):
                    _emit_rep(nc, pools, queues, tensors, ones, brow, sc,
                              single_q=single_q, half_mm=half_mm,
                              x_mode=x_mode, w_mode=w_mode,
                              out_mode=out_mode, out_eng=nc.gpsimd)

            if loops > 0:
                with tc.For_i(0, loops):
                    emit_reps()
            else:
                emit_reps()

    split_waits(nc)
    return nc


def shard_inputs(x, weight, weight_scale, bias):
    """Host-side marshalling into per-core input maps (layout + dtype only;
    the fp8 weight conversion is exact because the reference round-trips
    weight through fp8-e4m3)."""
    x = np.asarray(x, dtype=np.float32)
    weight = np.asarray(weight, dtype=np.float32)
    scale = np.asarray(weight_scale, dtype=np.float32).reshape(OUT)
    bias32 = np.asarray(bias).astype(np.float32)

    # pack x as [p, kt, m] (k = kt*128 + p) so each SBUF partition's x data
    # is one contiguous DRAM run
    xt = np.ascontiguousarray(
        np.transpose(x.reshape(M, KT, 128), (2, 1, 0))).astype(np.float16)
    in_maps = []
    for c in range(NCORES):
        sl = slice(c * OSH, (c + 1) * OSH)
        # wt[p, kt, o] = W_shard[o, kt*128 + p]  (k-major on partitions)
        wt = np.ascontiguousarray(
            weight[sl, :].T.reshape(KT, 128, OSH).transpose(1, 0, 2)
        ).astype(ml_dtypes.float8_e4m3)
        rows = np.concatenate(
            [scale[sl], bias32[sl] / scale[sl]])[None, :].astype(np.float16)
        in_maps.append({
            "xt": xt, "wt": wt, "rows": np.ascontiguousarray(rows),
        })
    return in_maps


def kernel(x, weight, weight_scale, bias):
    nc = build(reps=1)
    in_maps = shard_inputs(x, weight, weight_scale, bias)
    res = run_bass_kernel_spmd(nc, in_maps, core_ids=list(range(NCORES)))
    out = np.concatenate(
        [np.asarray(res.results[c]["out"]) for c in range(NCORES)], axis=1)
    return out.astype(np.float32).reshape(B, S, OUT)
